# revision 12
# baseline (speedup 1.0000x reference)
"""DCNv2 deformable ROI pooling on 8 Trainium2 NeuronCores.

Strategy: per-bin the 4x4 bilinear sample grid is separable (y outer-product
x), so each ROI's pooled output reduces to one small accumulated matmul
    out[49 bins, 256 ch] = M[49, K] @ PatchFlat[K, 256]
where K = R*L is a flattened feature-map patch window covering the ROI's
samples and M = alpha (x) beta is built from host-precomputed per-axis
interpolation weights.  ROIs (dim 0) are sharded across the 8 cores; the
channels-last feature map is replicated (bf16).

Slot layout (shared by all 8 cores so one NEFF runs SPMD): the 512 ROIs are
sorted by (row-span, col-span), stratified, and dealt into 64 groups of 8 —
one ROI per core per slot.  Each slot's patch window (R, Lp) is the max span
of its 8 ROIs, with the col-group factor G chosen per slot to minimize
padded width Lp (partition p = r*G + s holds pixels (row r, col s*nk + k)
for chunk k; Q = G*R partitions, nk = Lp/G chunks).

Per-ROI patch addresses are runtime data: they are DMA'd to SBUF once and
batch-loaded into engine registers in waves (one TENSOR_LOAD per wave of 8
instead of one per slot), then consumed by dynamic-offset patch DMAs that
alternate between the SP and ACT HWDGE rings.
"""

import numpy as np

import concourse.bass as bass
import concourse.mybir as mybir
import concourse.tile as tile
from concourse import bacc
import concourse.bass_utils as bass_utils

B, C, H, W = 4, 256, 128, 128
N_ROIS = 512
P = 7
PP = P * P
SCALE = np.float32(0.0625)
RATIO = 4
GAMMA = np.float32(0.1)
N_CORES = 8

MAX_NK = 6        # max matmul chunks per slot
OUT_GROUP = 16    # slots per packed output flush
WAVE = 8          # offset registers batch-loaded per TENSOR_LOAD
PATCH_BUFS = 8
PSUM_BUFS = 8

_f32 = np.float32


def _prep(rois, offset):
    """Dense per-axis interpolation weights + per-ROI sample bounds.

    Returns (bidx, ymin, ymax, xmin, xmax, alpha_d[N,PP,H], beta_d[N,PP,W]).
    """
    n = rois.shape[0]
    bidx = rois[:, 0].astype(np.int32)
    x1 = rois[:, 1] * SCALE - _f32(0.5)
    y1 = rois[:, 2] * SCALE - _f32(0.5)
    x2 = rois[:, 3] * SCALE - _f32(0.5)
    y2 = rois[:, 4] * SCALE - _f32(0.5)
    rw = np.maximum(x2 - x1, _f32(1.0))
    rh = np.maximum(y2 - y1, _f32(1.0))
    bw = rw / _f32(P)
    bh = rh / _f32(P)
    off = offset.reshape(n, 2, P, P).astype(np.float32)
    off_x = GAMMA * rw[:, None, None] * off[:, 0]
    off_y = GAMMA * rh[:, None, None] * off[:, 1]
    ph = np.arange(P, dtype=np.float32)
    s = ((np.arange(RATIO, dtype=np.float32) + _f32(0.5)) / _f32(RATIO))
    # mirror reference.py op order exactly (float32)
    ybase = y1[:, None, None] + ph[None, :, None] * bh[:, None, None] + off_y
    xbase = x1[:, None, None] + ph[None, None, :] * bw[:, None, None] + off_x
    ys = ybase[..., None] + s[None, None, None, :] * bh[:, None, None, None]
    xs = xbase[..., None] + s[None, None, None, :] * bw[:, None, None, None]
    vy = (ys > -1.0) & (ys < H)
    vx = (xs > -1.0) & (xs < W)
    yc = np.clip(ys, _f32(0.0), _f32(H - 1))
    xc = np.clip(xs, _f32(0.0), _f32(W - 1))
    y0 = np.floor(yc).astype(np.int32)
    x0 = np.floor(xc).astype(np.int32)
    y1i = np.minimum(y0 + 1, H - 1)
    x1i = np.minimum(x0 + 1, W - 1)
    ly = (yc - y0).astype(np.float32)
    lx = (xc - x0).astype(np.float32)
    hy = _f32(1.0) - ly
    hx = _f32(1.0) - lx

    npp = n * PP
    alpha_d = np.zeros((npp, H), np.float32)
    beta_d = np.zeros((npp, W), np.float32)
    rows = np.repeat(np.arange(npp), RATIO)
    inv = _f32(1.0 / RATIO)
    np.add.at(alpha_d, (rows, y0.reshape(npp, RATIO).ravel()),
              (np.where(vy, hy, 0).reshape(npp, RATIO) * inv).ravel())
    np.add.at(alpha_d, (rows, y1i.reshape(npp, RATIO).ravel()),
              (np.where(vy, ly, 0).reshape(npp, RATIO) * inv).ravel())
    np.add.at(beta_d, (rows, x0.reshape(npp, RATIO).ravel()),
              (np.where(vx, hx, 0).reshape(npp, RATIO) * inv).ravel())
    np.add.at(beta_d, (rows, x1i.reshape(npp, RATIO).ravel()),
              (np.where(vx, lx, 0).reshape(npp, RATIO) * inv).ravel())

    ymin = np.minimum(y0.reshape(n, -1).min(axis=1), H - 1)
    ymax = np.minimum(y1i.reshape(n, -1).max(axis=1), H - 1)
    xmin = np.minimum(x0.reshape(n, -1).min(axis=1), W - 1)
    xmax = np.minimum(x1i.reshape(n, -1).max(axis=1), W - 1)
    return (bidx, ymin, ymax, xmin, xmax,
            alpha_d.reshape(n, PP, H), beta_d.reshape(n, PP, W))


def _best_G(R, L):
    """Pick the col-group factor G minimizing padded width Lp (ties: fewer
    chunks nk, then more partitions).  Returns (G, Lp, nk).

    The HWDGE deals a strided transfer's work to SDMA engines per SOURCE
    ROW (engine = row index mod 16-ish), but any row's bytes beyond ~8 KB
    all land on engine 0, serializing the kernel.  Callers must keep
    rows <= 8 KB, i.e. Lp <= 16 f32-columns (split wider windows)."""
    best = None
    for G in range(1, 128 // R + 1):
        nk = -(-L // G)
        Lp = nk * G
        cand = (Lp > 16, nk > MAX_NK, Lp, nk, -G)
        if best is None or cand < best:
            best = cand
            bG = G
    nk = -(-L // bG)
    return bG, nk * bG, nk


def _fetch_split(R, L):
    """Split a window of width L into column segments of <= 16, each a
    separate fetch (R, Lseg, G, Lp, nk, col_off)."""
    segs = []
    n_seg = -(-L // 16)
    base = L // n_seg
    rem = L - base * n_seg
    col = 0
    for j in range(n_seg):
        Ls = base + (1 if j < rem else 0)
        G, Lp, nk = _best_G(R, Ls)
        segs.append((R, Ls, G, Lp, nk, col))
        col += Ls
    return segs


def _mt_block(alpha_w, beta_w, R, Lp, G, nk):
    """[PP, R] x [PP, Lp] weights -> device MT block [Q, nk*PP]."""
    Q = G * R
    p = np.arange(Q)
    a = alpha_w[:, p // G]                        # [PP, Q]
    l_idx = (p[:, None] % G) * nk + np.arange(nk)[None, :]   # [Q, nk]
    b = beta_w[:, l_idx]                          # [PP, Q, nk]
    mt = a.T[:, None, :] * b.transpose(1, 2, 0)   # [Q, nk, PP]
    return mt.reshape(Q, nk * PP).astype(np.float32)


def _layout_meta(fetches):
    """Free-dim offsets of each fetch's MT block in the resident SBUF tile,
    total free size, and the <=4 load-chunk split points (fetch-aligned)."""
    fo = []
    f = 0
    for (slot, R, Lp, G, nk) in fetches:
        fo.append(f)
        f += nk * PP
    bounds = fo + [f]
    n_chunks = 4
    splits = [0]
    for j in range(1, n_chunks):
        target = f * j // n_chunks
        splits.append(min(bounds, key=lambda b_: abs(b_ - target)))
    splits.append(f)
    splits = sorted(set(splits))
    return fo, f, splits


_NC_CACHE = {}


def _build_kernel(layout):
    """layout: per output slot, a tuple of fetch descriptors (R, Lp, G, nk);
    identical on every core.  A slot's fetches accumulate into one PSUM
    region; two slots share each 2 KB PSUM bank."""
    key = tuple(layout)
    if key in _NC_CACHE:
        return _NC_CACHE[key]
    n_slots = len(layout)
    fetches = [(s, R, Lp, G, nk)
               for s, segs in enumerate(layout)
               for (R, Lp, G, nk) in segs]
    n_fetch = len(fetches)
    fo, mt_free, splits = _layout_meta(fetches)
    first_f = {}
    for fi, (s, *_rest) in enumerate(fetches):
        first_f.setdefault(s, fi)
    data_dt = mybir.dt.bfloat16

    nc = bacc.Bacc("TRN2", target_bir_lowering=False, debug=False,
                   num_devices=N_CORES)
    xt = nc.dram_tensor("xt", [B, H, W, C], data_dt,
                        kind="ExternalInput").ap()
    mt = nc.dram_tensor("mt", [128, mt_free], data_dt,
                        kind="ExternalInput").ap()
    po = nc.dram_tensor("po", [1, n_fetch], mybir.dt.int32,
                        kind="ExternalInput").ap()
    n_groups = -(-n_slots // OUT_GROUP)
    # group-major output: out[g, b, s*C + c] holds slot g*OUT_GROUP+s
    out = nc.dram_tensor("out", [n_groups, PP, OUT_GROUP * C],
                         data_dt, kind="ExternalOutput").ap()

    # ring of fetch i: alternate per 8-block so offset register loads batch
    # into one TENSOR_LOAD per contiguous block
    rings = [(i // WAVE) % 2 for i in range(n_fetch)]

    with tile.TileContext(nc) as tc:
        with (
            tc.tile_pool(name="offp", bufs=1) as offp,
            tc.tile_pool(name="mtp", bufs=1) as mtp,
            tc.tile_pool(name="patchp", bufs=PATCH_BUFS) as patchp,
            tc.tile_pool(name="outp", bufs=3) as outp,
            tc.tile_pool(name="psump", bufs=PSUM_BUFS, space="PSUM") as psump,
        ):
            offs = offp.tile([1, n_fetch], mybir.dt.int32)
            nc.sync.dma_start(offs[:, :], po[:, :])
            mt_sb = mtp.tile([128, mt_free], data_dt)
            for ci, (a, b_) in enumerate(zip(splits[:-1], splits[1:])):
                issuer = nc.sync if ci % 2 == 0 else nc.scalar
                issuer.dma_start(mt_sb[:, a:b_], mt[:, a:b_])

            # patch offsets are batch-loaded into engine registers one
            # 8-block wave at a time (one TENSOR_LOAD per wave), lazily so
            # only ~2 waves of registers are live at once
            eng_of = {0: mybir.EngineType.SP, 1: mybir.EngineType.Activation}
            off_vals = [None] * n_fetch

            def load_wave(fi):
                lo = (fi // WAVE) * WAVE
                hi = min(lo + WAVE, n_fetch)
                max_ext = max(((R - 1) * W + Lp) * C
                              for (s, R, Lp, G, nk) in fetches[lo:hi])
                _, vals = nc.values_load_multi_w_load_instructions(
                    offs[0:1, lo:hi], engines=[eng_of[rings[fi]]],
                    min_val=0, max_val=B * H * W * C - max_ext,
                    skip_runtime_bounds_check=True)
                for i, v in zip(range(lo, hi), vals):
                    off_vals[i] = v

            fi = 0
            for g0 in range(0, n_slots, OUT_GROUP):
                gs = min(OUT_GROUP, n_slots - g0)
                osb = outp.tile([PP, OUT_GROUP * C], data_dt, tag="osb")
                if gs < OUT_GROUP:
                    nc.vector.memset(osb[:, gs * C:], 0.0)
                ps = None
                for i in range(g0, g0 + gs):
                    segs = layout[i]
                    # two slots share one 2 KB PSUM bank; drain them together
                    half = (i - g0) % 2
                    if half == 0:
                        ps = psump.tile([PP, 2 * C], mybir.dt.float32,
                                        space="PSUM")
                    n_seg = len(segs)
                    for j, (R, Lp, G, nk) in enumerate(segs):
                        Q = G * R
                        patch = patchp.tile([Q, nk * C], data_dt, tag="patch")
                        if off_vals[fi] is None:
                            load_wave(fi)
                        issuer = nc.sync if rings[fi] == 0 else nc.scalar
                        src = bass.AP(xt.tensor, off_vals[fi],
                                      [[W * C, R], [1, Lp * C]])
                        issuer.dma_start(patch[:, :], src)
                        for k in range(nk):
                            nc.tensor.matmul(
                                ps[:, half * C:(half + 1) * C],
                                lhsT=mt_sb[0:Q,
                                           fo[fi] + k * PP:fo[fi] + (k + 1) * PP],
                                rhs=patch[:, k * C:(k + 1) * C],
                                start=(j == 0 and k == 0),
                                stop=(j == n_seg - 1 and k == nk - 1))
                        fi += 1
                    s = i - g0
                    if half == 1 or i == g0 + gs - 1:
                        s0 = s - half
                        nc.vector.tensor_copy(
                            osb[:, s0 * C:(s + 1) * C],
                            ps[:, 0:(half + 1) * C])
                nc.gpsimd.dma_start(out[g0 // OUT_GROUP], osb[:, :])
    nc.compile()
    _NC_CACHE[key] = nc
    return nc


def _reference_fallback(x, rois, offset, idx):
    """Exact numpy replica of the reference (safety net for ROIs whose span
    exceeds every packable window; unused for the benchmark distribution)."""
    n = len(idx)
    if n == 0:
        return np.zeros((0, C, P, P), np.float32)
    rois = rois[idx]
    offset = offset[idx]
    bidx = rois[:, 0].astype(np.int32)
    x1 = rois[:, 1] * SCALE - _f32(0.5)
    y1 = rois[:, 2] * SCALE - _f32(0.5)
    x2 = rois[:, 3] * SCALE - _f32(0.5)
    y2 = rois[:, 4] * SCALE - _f32(0.5)
    rw = np.maximum(x2 - x1, _f32(1.0))
    rh = np.maximum(y2 - y1, _f32(1.0))
    bw, bh = rw / _f32(P), rh / _f32(P)
    off = offset.reshape(n, 2, P, P)
    off_x = GAMMA * rw[:, None, None] * off[:, 0]
    off_y = GAMMA * rh[:, None, None] * off[:, 1]
    ph = np.arange(P, dtype=np.float32)
    s = (np.arange(RATIO, dtype=np.float32) + _f32(0.5)) / _f32(RATIO)
    ybase = y1[:, None, None] + ph[None, :, None] * bh[:, None, None] + off_y
    xbase = x1[:, None, None] + ph[None, None, :] * bw[:, None, None] + off_x
    ys = ybase[..., None, None] + s[:, None][None, None, None] * bh[:, None, None, None, None]
    xs = xbase[..., None, None] + s[None, :][None, None, None] * bw[:, None, None, None, None]
    ys, xs = np.broadcast_arrays(ys, xs)
    valid = (ys > -1.0) & (ys < H) & (xs > -1.0) & (xs < W)
    yc = np.clip(ys, 0.0, _f32(H - 1))
    xc = np.clip(xs, 0.0, _f32(W - 1))
    y0 = np.floor(yc).astype(np.int32)
    x0 = np.floor(xc).astype(np.int32)
    y1i = np.minimum(y0 + 1, H - 1)
    x1i = np.minimum(x0 + 1, W - 1)
    ly = (yc - y0).astype(np.float32)
    lx = (xc - x0).astype(np.float32)
    hy, hx = _f32(1.0) - ly, _f32(1.0) - lx
    b = bidx[:, None, None, None, None]
    val = ((hy * hx)[..., None] * x[b, :, y0, x0]
           + (hy * lx)[..., None] * x[b, :, y0, x1i]
           + (ly * hx)[..., None] * x[b, :, y1i, x0]
           + (ly * lx)[..., None] * x[b, :, y1i, x1i])
    val = np.where(valid[..., None], val, _f32(0.0))
    return val.mean(axis=(3, 4)).transpose(0, 3, 1, 2)


def kernel(input, rois, offset):
    import ml_dtypes
    input = np.asarray(input, dtype=np.float32)
    rois = np.asarray(rois, dtype=np.float32)
    offset = np.asarray(offset, dtype=np.float32)

    xt = np.ascontiguousarray(
        input.transpose(0, 2, 3, 1)).astype(ml_dtypes.bfloat16)
    bidx, ymin, ymax, xmin, xmax, alpha_d, beta_d = _prep(rois, offset)
    n = rois.shape[0]
    sr = (ymax - ymin + 1).astype(int)
    sl = (xmax - xmin + 1).astype(int)

    # ROIs whose span can't fit any packable window go to the numpy fallback
    ok = [i for i in range(n) if sr[i] <= 128 and sl[i] <= 128]
    fallback_idx = [i for i in range(n) if i not in set(ok)]
    ok = np.array(ok, np.int64)
    # pad to a multiple of 8 ROIs by repeating the first (output discarded)
    n_ok = len(ok)
    pad = (-n_ok) % N_CORES
    okp = np.concatenate([ok, np.repeat(ok[:1], pad)]) if pad else ok

    # sort by (row-span, col-span); stratify; deal groups of 8 (one per core)
    order = okp[np.argsort(sr[okp] * 1000 + sl[okp], kind="stable")]
    n_slots = len(order) // N_CORES
    strat = max(1, min(8, n_slots // 8))
    per = -(-n_slots // strat) * N_CORES
    chunks = [order[i * per:(i + 1) * per] for i in range(strat)]
    groups_r = []
    for ch in chunks:
        ch = ch[np.argsort(sl[ch], kind="stable")]
        groups_r += [ch[j * N_CORES:(j + 1) * N_CORES]
                     for j in range(len(ch) // N_CORES)]
    assert len(groups_r) == n_slots

    layout = []        # per slot: tuple of (R, Lp, G, nk)
    slot_segs = []     # per slot: list of (R, Ls, G, Lp, nk, col_off)
    slot_roi = np.zeros((N_CORES, n_slots), np.int64)
    for g, grp in enumerate(groups_r):
        R = int(sr[grp].max())
        L = int(sl[grp].max())
        segs = _fetch_split(R, L)
        slot_segs.append(segs)
        layout.append(tuple((R_, Lp, G, nk) for (R_, Ls, G, Lp, nk, co) in segs))
        slot_roi[:, g] = grp
    layout = tuple(layout)
    fetches = [(s, R, Lp, G, nk)
               for s, segs in enumerate(layout)
               for (R, Lp, G, nk) in segs]
    n_fetch = len(fetches)
    fo, mt_free, _ = _layout_meta(fetches)

    # build per-core inputs
    mt_all = np.zeros((N_CORES, 128, mt_free), ml_dtypes.bfloat16)
    po_all = np.zeros((N_CORES, n_fetch), np.int32)
    for core in range(N_CORES):
        fi = 0
        for slot in range(n_slots):
            ridx = int(slot_roi[core, slot])
            for (R, Ls, G, Lp, nk, col_off) in slot_segs[slot]:
                py0 = min(max(int(ymin[ridx]), 0), H - R)
                x_lo = min(max(int(xmin[ridx]), 0), W - 1) + col_off
                px0 = min(x_lo, W - Lp)
                # beta weights for this fetch: only columns inside the
                # segment's true range [x_lo, x_lo + Ls)
                bw = np.zeros((PP, Lp), np.float32)
                for k in range(Lp):
                    gx = px0 + k
                    if x_lo <= gx < min(x_lo + Ls, W):
                        bw[:, k] = beta_d[ridx, :, gx]
                blk = _mt_block(alpha_d[ridx, :, py0:py0 + R],
                                bw, R, Lp, G, nk)
                mt_all[core, 0:G * R, fo[fi]:fo[fi] + nk * PP] = blk
                po_all[core, fi] = ((int(bidx[ridx]) * H + py0) * W + px0) * C
                fi += 1

    nc = _build_kernel(layout)
    in_maps = [{"xt": xt, "mt": mt_all[c], "po": po_all[c][None, :]}
               for c in range(N_CORES)]
    kernel.last_nc = nc
    kernel.last_in_maps = in_maps
    runner = getattr(kernel, "runner", None)
    if runner is not None:
        res = runner(nc, in_maps)
    else:
        res = bass_utils.run_bass_kernel_spmd(nc, in_maps,
                                              core_ids=list(range(N_CORES)))
    kernel.last_results = res

    out = np.zeros((n, C, P, P), np.float32)
    for core in range(N_CORES):
        dev = res.results[core]["out"]     # [n_groups, PP, OUT_GROUP*C] bf16
        for slot in range(n_slots):
            ridx = int(slot_roi[core, slot])
            g, s = divmod(slot, OUT_GROUP)
            out[ridx] = dev[g][:, s * C:(s + 1) * C].astype(
                np.float32).T.reshape(C, P, P)

    if fallback_idx:
        out[fallback_idx] = _reference_fallback(input, rois, offset,
                                                np.array(fallback_idx))
    return np.ascontiguousarray(out)
